# revision 1
# baseline (speedup 1.0000x reference)
"""Trainium2 Bass kernel for CustomSimplexMappingAttention (sparsemax attention).

Sharding: batch*head parallel across 8 cores. Core c handles batch b=c//4 and
heads {2*(c%4), 2*(c%4)+1}. Each core computes its two heads' full attention
plus its partial contribution to the output projection; the host sums partials.

Per-core algorithm (per head):
  scores tile [128 q, W] (causal W=128*(qi+1)) in PSUM via fp32r matmuls
  candidate extraction: top-8 of each quarter-segment via DVE max8 (exact fp32)
  per-tile shift m (max of cands); ACT relu(z-(m-1)) evacuates PSUM -> fp16 t
  sorted top-16 per row (max8+match_replace on the 32 candidates), cumsum,
  closed-form sparsemax threshold tau (all fp32, exact)
  attn = relu(t - (tau-m+1)) fp16 (DVE), DMA-transposed into k-major layout,
  fp16 matmuls attn.T @ v accumulate out.T; output projection partial in fp32r.
"""

import os
from contextlib import ExitStack

import numpy as np

import concourse.bass as bass
import concourse.tile as tile
from concourse import bacc
from concourse import mybir
from concourse.bass_utils import run_bass_kernel_spmd

F32 = mybir.dt.float32
F32R = mybir.dt.float32r
F16 = mybir.dt.float16

P = 128
L = 2048
D = 512
HD = 64
NT = L // P  # 16 q tiles
NEG = -1e9
NSEG = 4   # candidate segments per row
NC8 = 8 * NSEG  # 32 raw candidates
NCAND = 16  # sorted candidates kept


def _build_program(dbg=False):
    nc = bacc.Bacc("TRN2", target_bir_lowering=False, debug=False)

    xt = nc.dram_tensor("xt", [D, L], F32R, kind="ExternalInput").ap()
    wq = nc.dram_tensor("wq", [D, P], F32R, kind="ExternalInput").ap()
    wk = nc.dram_tensor("wk", [D, P], F32R, kind="ExternalInput").ap()
    wv = nc.dram_tensor("wv", [D, P], F32R, kind="ExternalInput").ap()
    wo = nc.dram_tensor("wo", [P, D], F32R, kind="ExternalInput").ap()
    dmask = nc.dram_tensor("dmask", [P, P], F32R, kind="ExternalInput").ap()
    ident = nc.dram_tensor("ident", [P, P], F32R, kind="ExternalInput").ap()
    identh = nc.dram_tensor("identh", [P, P], F16, kind="ExternalInput").ap()
    kvec = nc.dram_tensor("kvec", [P, NT * NCAND], F32, kind="ExternalInput").ap()
    outT = nc.dram_tensor("outT", [D, L], F32, kind="ExternalOutput").ap()
    dbg_aps = None
    if dbg:
        dbg_aps = {
            "d_qT2": nc.dram_tensor("d_qT2", [P, L], F32, kind="ExternalOutput").ap(),
            "d_kT2": nc.dram_tensor("d_kT2", [P, L], F32, kind="ExternalOutput").ap(),
            "d_v2": nc.dram_tensor("d_v2", [P, L], F32, kind="ExternalOutput").ap(),
            "d_cand": nc.dram_tensor("d_cand", [P, NT * NC8], F32, kind="ExternalOutput").ap(),
            "d_sort16": nc.dram_tensor("d_sort16", [P, NT * NCAND], F32, kind="ExternalOutput").ap(),
            "d_tau": nc.dram_tensor("d_tau", [P, NT], F32, kind="ExternalOutput").ap(),
            "d_attn": nc.dram_tensor("d_attn", [P, 3 * P], F32, kind="ExternalOutput").ap(),
            "d_attnT": nc.dram_tensor("d_attnT", [P, 3 * P], F32, kind="ExternalOutput").ap(),
            "d_hoT2": nc.dram_tensor("d_hoT2", [P, L], F32, kind="ExternalOutput").ap(),
        }
    with tile.TileContext(nc) as tc:
        _kernel_body(tc, outT, xt, wq, wk, wv, wo, dmask, ident, identh, kvec, dbg_aps)
    nc.finalize()
    return nc


def _kernel_body(tc, outT, xt, wq, wk, wv, wo, dmask, ident, identh, kvec, dbg_aps=None):
    nc = tc.nc
    Relu = mybir.ActivationFunctionType.Relu
    Copy = mybir.ActivationFunctionType.Copy
    Alu = mybir.AluOpType

    with ExitStack() as ctx:
        consts = ctx.enter_context(tc.tile_pool(name="consts", bufs=1))
        dmask_sb = consts.tile([P, P], F32R)
        nc.sync.dma_start(dmask_sb[:], dmask)
        ident_sb = consts.tile([P, P], F32R)
        nc.sync.dma_start(ident_sb[:], ident)
        identh_sb = consts.tile([P, P], F16)
        nc.sync.dma_start(identh_sb[:], identh)
        kvec_sb = consts.tile([P, NT * NCAND], F32)
        nc.sync.dma_start(kvec_sb[:], kvec)

        # persistent activation tiles
        persist = ctx.enter_context(tc.tile_pool(name="persist", bufs=1))
        qT2 = persist.tile([P, L], F32R)   # q.T both heads [i(2h), l]
        kT2 = persist.tile([P, L], F32R)
        v2 = persist.tile([P, L], F16)     # v chunks: block c cols -> [n in c, i(2h)]
        hoT2 = persist.tile([P, L], F32R)  # head outs .T, head h rows 64h:64h+64

        # ---------------- projections ----------------
        with ExitStack() as pctx:
            xpool = pctx.enter_context(tc.tile_pool(name="xt", bufs=1))
            wpool = pctx.enter_context(tc.tile_pool(name="w", bufs=1))
            ppool = pctx.enter_context(tc.tile_pool(name="pproj", bufs=1, space="PSUM"))
            tpool = pctx.enter_context(tc.tile_pool(name="ptrans", bufs=4, space="PSUM"))
            vtpool = pctx.enter_context(tc.tile_pool(name="vt", bufs=1))

            xt_sb = [xpool.tile([P, L], F32R, tag=f"xt{i}", name=f"xt{i}") for i in range(4)]
            for kc in range(4):
                nc.sync.dma_start(xt_sb[kc][:], xt[P * kc:P * (kc + 1), :])
            w_sb = {}
            for name, w in (("q", wq), ("k", wk), ("v", wv)):
                t = wpool.tile([P, 4 * P], F32R, tag=f"w{name}")
                nc.sync.dma_start(
                    t.rearrange("p (c i) -> p c i", i=P),
                    w.rearrange("(c p) i -> p c i", p=P),
                )
                w_sb[name] = t

            vT2_f16 = vtpool.tile([P, L], F16)
            for name, dst in (("q", qT2), ("k", kT2), ("v", None)):
                ps = ppool.tile([P, L], F32, tag="projps")
                for nc_i in range(4):
                    nsl = slice(512 * nc_i, 512 * (nc_i + 1))
                    for kc in range(4):
                        nc.tensor.matmul(
                            ps[:, nsl],
                            w_sb[name][:, P * kc:P * (kc + 1)],
                            xt_sb[kc][:, nsl],
                            start=(kc == 0), stop=(kc == 3),
                        )
                if dst is not None:
                    nc.scalar.activation(dst[:], ps[:], Copy)
                else:
                    nc.scalar.activation(vT2_f16[:], ps[:], Copy)

            # transpose vT2 [i, n] -> v2 chunks [n, i], batched evacuation
            for g in range(0, NT, 4):
                pt = tpool.tile([P, 4 * P], F16, tag="vtr", name=f"vtr{g}")
                for c in range(g, g + 4):
                    nc.tensor.transpose(
                        pt[:, P * (c - g):P * (c - g + 1)],
                        vT2_f16[:, P * c:P * (c + 1)], identh_sb[:])
                nc.vector.tensor_copy(v2[:, P * g:P * (g + 4)], pt[:])
            if dbg_aps is not None:
                nc.gpsimd.dma_start(dbg_aps["d_qT2"], qT2[:])
                nc.gpsimd.dma_start(dbg_aps["d_kT2"], kT2[:])
                dv2 = vtpool.tile([P, L], F32, name="dv2")
                nc.vector.tensor_copy(dv2[:], v2[:])
                nc.sync.dma_start(dbg_aps["d_v2"], dv2[:])

        # ---------------- attention (per head) ----------------
        with ExitStack() as actx:
            spool = actx.enter_context(tc.tile_pool(name="spsum", bufs=2, space="PSUM"))
            opool = actx.enter_context(tc.tile_pool(name="opsum", bufs=1, space="PSUM"))
            tpsum = actx.enter_context(tc.tile_pool(name="tpsum", bufs=2, space="PSUM"))
            apool = actx.enter_context(tc.tile_pool(name="attn", bufs=1))
            atpool = actx.enter_context(tc.tile_pool(name="attnT", bufs=1))
            cpool = actx.enter_context(tc.tile_pool(name="cands", bufs=1))
            smpool = actx.enter_context(tc.tile_pool(name="smalls", bufs=2))

            for h in range(2):
                hsl = slice(HD * h, HD * (h + 1))

                cand = cpool.tile([P, NT * NC8], F32, tag="cand")
                sort16 = cpool.tile([P, NT * NCAND], F32, tag="sort16")
                mrow = cpool.tile([P, NT], F32, tag="mrow")
                bias1 = cpool.tile([P, NT], F32, tag="bias1")
                # attn tiles (fp16), also used as t (shifted scores)
                attn_t = [apool.tile([P, P * (qi + 1)], F16, tag=f"at{qi}", name=f"at{h}_{qi}")
                          for qi in range(NT)]
                attnT_t = [atpool.tile([P, P * (qi + 1)], F16, tag=f"aT{qi}", name=f"aT{h}_{qi}")
                           for qi in range(NT)]

                for qi in range(NT):
                    W = P * (qi + 1)
                    qsl = slice(P * qi, P * (qi + 1))
                    csl = lambda s: slice(qi * NC8 + 8 * s, qi * NC8 + 8 * (s + 1))
                    for half in range(2):
                        hw = W // 2
                        # overlap of diag block [W-P, W) with this half, in
                        # half-local coords
                        mlo = max(0, (W - P) - half * hw)
                        mhi = min(hw, W - half * hw)
                        has_mask = mhi > mlo
                        ps = spool.tile([P, 1024], F32, tag="sps")
                        nchunks = (hw + 511) // 512
                        for ncx in range(nchunks):
                            n0 = 512 * ncx
                            n1 = min(hw, n0 + 512)
                            nc.tensor.matmul(
                                ps[:, n0:n1],
                                qT2[hsl, qsl],
                                kT2[hsl, half * hw + n0:half * hw + n1],
                                start=True, stop=True,
                            )
                        if has_mask:
                            # additive causal mask on (part of) the diag block
                            dlo = mlo + half * hw - (W - P)
                            dhi = mhi + half * hw - (W - P)
                            nc.tensor.matmul(
                                ps[:, mlo:mhi],
                                ident_sb[:],
                                dmask_sb[:, dlo:dhi],
                                start=False, stop=True,
                                skip_group_check=True,
                            )
                        # candidates: top-8 of each quarter (2 per half)
                        for s in range(2):
                            seg = s + 2 * half
                            nc.vector.max(
                                out=cand[:, csl(seg)],
                                in_=ps[:, s * (hw // 2):(s + 1) * (hw // 2)],
                            )
                        if half == 0:
                            ps0 = ps
                        else:
                            # row shift m = max of the 4 segment heads
                            nc.vector.tensor_reduce(
                                mrow[:, qi:qi + 1],
                                cand[:, qi * NC8:(qi + 1) * NC8].rearrange(
                                    "p (s e) -> p s e", e=8)[:, :, 0:1],
                                axis=mybir.AxisListType.XY, op=Alu.max,
                            )
                            # bias1 = 1 - m
                            nc.vector.tensor_scalar(
                                out=bias1[:, qi:qi + 1], in0=mrow[:, qi:qi + 1],
                                scalar1=-1.0, scalar2=1.0,
                                op0=Alu.mult, op1=Alu.add,
                            )
                            # evacuate both halves: t = relu(z - (m-1)) -> fp16
                            nc.scalar.activation(
                                attn_t[qi][:, 0:hw], ps0[:, 0:hw], Relu,
                                bias=bias1[:, qi:qi + 1],
                            )
                            nc.scalar.activation(
                                attn_t[qi][:, hw:W], ps[:, 0:hw], Relu,
                                bias=bias1[:, qi:qi + 1],
                            )

                # sorted top-16 of the 32 candidates, per tile
                for qi in range(NT):
                    c32 = cand[:, qi * NC8:(qi + 1) * NC8]
                    s16 = sort16[:, qi * NCAND:(qi + 1) * NCAND]
                    scr = smpool.tile([P, NC8], F32, tag="scr")
                    nc.vector.max(out=s16[:, 0:8], in_=c32)
                    nc.vector.match_replace(
                        out=scr[:], in_to_replace=s16[:, 0:8], in_values=c32,
                        imm_value=NEG,
                    )
                    nc.vector.max(out=s16[:, 8:16], in_=scr[:])

                # stacked tau computation (fp32, exact): view [P, NT, NCAND]
                s3 = sort16.rearrange("p (t c) -> p t c", c=NCAND)
                cum = smpool.tile([P, NT * NCAND], F32, tag="cum")
                cum3 = cum.rearrange("p (t c) -> p t c", c=NCAND)
                nc.vector.tensor_copy(cum[:], sort16[:])
                tmp = smpool.tile([P, NT * NCAND], F32, tag="tmp")
                tmp3 = tmp.rearrange("p (t c) -> p t c", c=NCAND)
                src, dst = cum3, tmp3
                srcf, dstf = cum, tmp
                for d in (1, 2, 4, 8):
                    nc.vector.tensor_tensor(
                        out=dst[:, :, d:], in0=src[:, :, d:], in1=src[:, :, :NCAND - d],
                        op=Alu.add,
                    )
                    nc.vector.tensor_copy(dst[:, :, 0:d], src[:, :, 0:d])
                    src, dst = dst, src
                    srcf, dstf = dstf, srcf
                # src now holds cumsum
                # cond = (1 + k*v - S) > 0
                u = smpool.tile([P, NT * NCAND], F32, tag="u")
                nc.vector.tensor_tensor(out=u[:], in0=sort16[:], in1=kvec_sb[:], op=Alu.mult)
                nc.vector.tensor_tensor(out=u[:], in0=u[:], in1=srcf[:], op=Alu.subtract)
                cnd = smpool.tile([P, NT * NCAND], F32, tag="cnd")
                nc.vector.tensor_scalar(
                    out=cnd[:], in0=u[:], scalar1=-1.0, scalar2=None, op0=Alu.is_gt,
                )
                # S_kz = sum(cnd * v); kz = sum(cnd)
                pv = smpool.tile([P, NT * NCAND], F32, tag="pv")
                nc.vector.tensor_tensor(out=pv[:], in0=cnd[:], in1=sort16[:], op=Alu.mult)
                skz = smpool.tile([P, NT], F32, tag="skz")
                nc.vector.tensor_reduce(
                    skz[:], pv.rearrange("p (t c) -> p t c", c=NCAND),
                    axis=mybir.AxisListType.X, op=Alu.add,
                )
                kz = smpool.tile([P, NT], F32, tag="kz")
                nc.vector.tensor_reduce(
                    kz[:], cnd.rearrange("p (t c) -> p t c", c=NCAND),
                    axis=mybir.AxisListType.X, op=Alu.add,
                )
                rkz = smpool.tile([P, NT], F32, tag="rkz")
                nc.vector.reciprocal(rkz[:], kz[:])
                # delta = (S-1)/kz - (m-1) ; store negdelta = -delta
                nc.vector.tensor_scalar(
                    out=skz[:], in0=skz[:], scalar1=-1.0, scalar2=None, op0=Alu.add,
                )
                tauz = smpool.tile([P, NT], F32, tag="tauz")
                nc.vector.tensor_tensor(out=tauz[:], in0=skz[:], in1=rkz[:], op=Alu.mult)
                ndelta = smpool.tile([P, NT], F32, tag="ndelta")
                # ndelta = (m - 1) - tau = -(bias1) ... bias1 = 1-m so m-1 = -bias1
                nc.vector.tensor_tensor(out=ndelta[:], in0=bias1[:], in1=tauz[:], op=Alu.add)
                nc.vector.tensor_scalar(
                    out=ndelta[:], in0=ndelta[:], scalar1=-1.0, scalar2=None, op0=Alu.mult,
                )

                if dbg_aps is not None and h == 0:
                    nc.sync.dma_start(dbg_aps["d_cand"], cand[:])
                    nc.sync.dma_start(dbg_aps["d_sort16"], sort16[:])
                    nc.sync.dma_start(dbg_aps["d_tau"], tauz[:])
                # attn = relu(t - delta), PE-transpose blocks, attn.T @ v
                for ohalf in range(1, -1, -1):
                    psum_o = opool.tile([HD, L // 2], F32, tag="po",
                                        name=f"po{h}_{ohalf}")
                    for qi in range(8 * ohalf + 7, 8 * ohalf - 1, -1):
                        nc.vector.tensor_scalar(
                            out=attn_t[qi][:], in0=attn_t[qi][:],
                            scalar1=ndelta[:, qi:qi + 1], scalar2=0.0,
                            op0=Alu.add, op1=Alu.max,
                        )
                        for g in range(0, qi + 1, 4):
                            gend = min(qi + 1, g + 4)
                            gw = P * (gend - g)
                            pt = tpsum.tile([P, 4 * P], F16, tag="atr",
                                            name=f"atr{h}_{qi}_{g}")
                            for c in range(g, gend):
                                nc.tensor.transpose(
                                    pt[:, P * (c - g):P * (c - g + 1)],
                                    attn_t[qi][:, P * c:P * (c + 1)],
                                    identh_sb[:])
                            nc.vector.tensor_copy(
                                attnT_t[qi][:, P * g:P * g + gw], pt[:, :gw])
                            for c in range(g, gend):
                                nc.tensor.matmul(
                                    psum_o[:, P * (qi - 8 * ohalf):P * (qi - 8 * ohalf + 1)],
                                    v2[:, P * c:P * (c + 1)][:, hsl],
                                    attnT_t[qi][:, P * c:P * (c + 1)],
                                    start=(c == 0), stop=(c == qi),
                                )
                    nc.scalar.activation(
                        hoT2[HD * h:HD * (h + 1),
                             (L // 2) * ohalf:(L // 2) * (ohalf + 1)],
                        psum_o[:], Copy)
                if dbg_aps is not None and h == 0:
                    da = smpool.tile([P, 3 * P], F32, name="da")
                    nc.vector.tensor_copy(da[:], attn_t[2][:])
                    nc.sync.dma_start(dbg_aps["d_attn"], da[:])
                    db = smpool.tile([P, 3 * P], F32, name="db")
                    nc.vector.tensor_copy(db[:], attnT_t[2][:])
                    nc.sync.dma_start(dbg_aps["d_attnT"], db[:])
                if dbg_aps is not None and h == 1:
                    nc.gpsimd.dma_start(dbg_aps["d_hoT2"], hoT2[:])

        # ---------------- output projection ----------------
        with ExitStack() as octx:
            wopool = octx.enter_context(tc.tile_pool(name="wo", bufs=1))
            opsum = octx.enter_context(tc.tile_pool(name="opj", bufs=2, space="PSUM"))
            ostage = octx.enter_context(tc.tile_pool(name="ost", bufs=2))
            wo_sb = wopool.tile([P, D], F32R)
            nc.sync.dma_start(wo_sb[:], wo)
            for jc in range(4):
                ps = opsum.tile([P, L], F32, tag="ops")
                for ncx in range(4):
                    nsl = slice(512 * ncx, 512 * (ncx + 1))
                    nc.tensor.matmul(
                        ps[:, nsl], wo_sb[:, P * jc:P * (jc + 1)], hoT2[:, nsl],
                        start=True, stop=True,
                    )
                ot = ostage.tile([P, L], F32, tag="ot")
                nc.scalar.activation(ot[:], ps[:], Copy)
                nc.sync.dma_start(outT[P * jc:P * (jc + 1), :], ot[:])


_NC_CACHE = {}


def _get_program():
    if "nc" not in _NC_CACHE:
        _NC_CACHE["nc"] = _build_program()
    return _NC_CACHE["nc"]


def kernel(x, W_q, W_k, W_v, W_o):
    x = np.asarray(x, dtype=np.float32)
    W_q = np.asarray(W_q, dtype=np.float32)
    W_k = np.asarray(W_k, dtype=np.float32)
    W_v = np.asarray(W_v, dtype=np.float32)
    W_o = np.asarray(W_o, dtype=np.float32)
    b = x.shape[0]

    dmask_np = np.where(
        np.arange(P)[None, :] > np.arange(P)[:, None], np.float32(NEG), np.float32(0.0)
    ).astype(np.float32)
    ident_np = np.eye(P, dtype=np.float32)
    identh_np = np.eye(P, dtype=np.float16)
    kvec_np = np.broadcast_to(
        np.tile(np.arange(1, NCAND + 1, dtype=np.float32), NT)[None, :], (P, NT * NCAND)
    ).copy()

    in_maps = []
    for c in range(8):
        bb = c // 4
        j2 = c % 4
        hs = slice(P * j2, P * (j2 + 1))
        in_maps.append({
            "xt": np.ascontiguousarray(x[bb].T),
            # 1/sqrt(head_dim) score scale folded into the q projection
            "wq": np.ascontiguousarray(W_q[hs].T) * np.float32(1.0 / 8.0),
            "wk": np.ascontiguousarray(W_k[hs].T),
            "wv": np.ascontiguousarray(W_v[hs].T),
            "wo": np.ascontiguousarray(W_o[:, hs].T),
            "dmask": dmask_np,
            "ident": ident_np,
            "identh": identh_np,
            "kvec": kvec_np,
        })

    nc = _get_program()
    res = run_bass_kernel_spmd(
        nc, in_maps, core_ids=list(range(8)),
        trace=bool(int(os.environ.get("KERNEL_TRACE", "0"))),
    )
    _NC_CACHE["last_results"] = res

    out = np.zeros((b, L, D), dtype=np.float32)
    for c in range(8):
        out[c // 4] += res.results[c]["outT"].T
    return out



# revision 3
# speedup vs baseline: 14.7762x; 14.7762x over previous
"""Trainium2 Bass kernel for CustomSimplexMappingAttention (sparsemax attention).

Sharding: batch*head parallel across 8 cores. Core c handles batch b=c//4 and
heads {2*(c%4), 2*(c%4)+1}. Each core computes its two heads' attention output
(pre output-projection); the host applies W_o.

Wall-clock is dominated by the axon host<->device tunnel, so the call is
structured to minimize transferred bytes and round trips:
  - one packed fp16 upload [8, 360448] (x sliced by seq + W_q/k/v sliced by row;
    every byte uploaded exactly once),
  - stage-1 XLA jit on the 8 cores: all_gather + per-core slicing/transposes,
    constants (causal mask, identity, k-vector) and the zero donation buffers
    generated on device,
  - stage-2: the Bass NEFF via the _bass_exec custom call on device-resident
    arrays (jit cached across calls),
  - one fp16 download [1024, 2048] of per-head outputs; host does the W_o
    projection (two ~0.5 GFLOP sgemms).

Per-core Bass algorithm (per head):
  scores tile [128 q, W] (causal W=128*(qi+1)) in PSUM via fp16 matmuls
  candidate extraction: top-8 of each quarter-segment via DVE max8 (exact fp32)
  per-tile shift m (max of cands); ACT relu(z-(m-1)) evacuates PSUM -> fp16 t
  sorted top-16 per row (max8+match_replace on the 32 candidates), cumsum,
  closed-form sparsemax threshold tau (all fp32, exact)
  attn = relu(t - (tau-m+1)) fp16 (DVE), PE-transposed into k-major layout,
  fp16 matmuls attn.T @ v accumulate out.T -> hoT2 fp16 -> single DMA out.
"""

import os
from contextlib import ExitStack

import numpy as np

import concourse.bass as bass
import concourse.tile as tile
from concourse import bacc
from concourse import mybir
from concourse import bass2jax as b2j
from concourse.bass_utils import run_bass_kernel_spmd

F32 = mybir.dt.float32
F16 = mybir.dt.float16

P = 128
L = 2048
D = 512
HD = 64
NT = L // P  # 16 q tiles
NEG = -60000.0  # fp16-representable stand-in for -inf in the causal mask
NSEG = 4   # candidate segments per row
NC8 = 8 * NSEG  # 32 raw candidates
NCAND = 16  # sorted candidates kept

XSZ = 512 * 512          # packed per-core x slab elements
WSZ = 64 * 512           # packed per-core weight slab elements (per weight)
NPK = XSZ + 3 * WSZ      # packed per-core upload row


def _build_program():
    nc = bacc.Bacc("TRN2", target_bir_lowering=False, debug=False)

    xt = nc.dram_tensor("xt", [D, L], F16, kind="ExternalInput").ap()
    wq = nc.dram_tensor("wq", [D, P], F16, kind="ExternalInput").ap()
    wk = nc.dram_tensor("wk", [D, P], F16, kind="ExternalInput").ap()
    wv = nc.dram_tensor("wv", [D, P], F16, kind="ExternalInput").ap()
    dmask = nc.dram_tensor("dmask", [P, P], F16, kind="ExternalInput").ap()
    identh = nc.dram_tensor("identh", [P, P], F16, kind="ExternalInput").ap()
    kvec = nc.dram_tensor("kvec", [P, NT * NCAND], F32, kind="ExternalInput").ap()
    ho = nc.dram_tensor("ho", [P, L], F16, kind="ExternalOutput").ap()
    with tile.TileContext(nc) as tc:
        _kernel_body(tc, ho, xt, wq, wk, wv, dmask, identh, kvec)
    nc.finalize()
    return nc


def _kernel_body(tc, ho, xt, wq, wk, wv, dmask, identh, kvec):
    nc = tc.nc
    Relu = mybir.ActivationFunctionType.Relu
    Copy = mybir.ActivationFunctionType.Copy
    Alu = mybir.AluOpType

    with ExitStack() as ctx:
        consts = ctx.enter_context(tc.tile_pool(name="consts", bufs=1))
        dmask_sb = consts.tile([P, P], F16)
        nc.sync.dma_start(dmask_sb[:], dmask)
        identh_sb = consts.tile([P, P], F16)
        nc.sync.dma_start(identh_sb[:], identh)
        kvec_sb = consts.tile([P, NT * NCAND], F32)
        nc.sync.dma_start(kvec_sb[:], kvec)

        # persistent activation tiles
        persist = ctx.enter_context(tc.tile_pool(name="persist", bufs=1))
        qT2 = persist.tile([P, L], F16)    # q.T both heads [i(2h), l]
        kT2 = persist.tile([P, L], F16)
        v2 = persist.tile([P, L], F16)     # v chunks: block c cols -> [n in c, i(2h)]
        hoT2 = persist.tile([P, L], F16)   # head outs .T, head h rows 64h:64h+64

        # ---------------- projections ----------------
        with ExitStack() as pctx:
            xpool = pctx.enter_context(tc.tile_pool(name="xt", bufs=1))
            wpool = pctx.enter_context(tc.tile_pool(name="w", bufs=1))
            ppool = pctx.enter_context(tc.tile_pool(name="pproj", bufs=1, space="PSUM"))
            tpool = pctx.enter_context(tc.tile_pool(name="ptrans", bufs=4, space="PSUM"))
            vtpool = pctx.enter_context(tc.tile_pool(name="vt", bufs=1))

            xt_sb = [xpool.tile([P, L], F16, tag=f"xt{i}", name=f"xt{i}") for i in range(4)]
            for kc in range(4):
                nc.sync.dma_start(xt_sb[kc][:], xt[P * kc:P * (kc + 1), :])
            w_sb = {}
            for name, w in (("q", wq), ("k", wk), ("v", wv)):
                t = wpool.tile([P, 4 * P], F16, tag=f"w{name}")
                nc.sync.dma_start(
                    t.rearrange("p (c i) -> p c i", i=P),
                    w.rearrange("(c p) i -> p c i", p=P),
                )
                w_sb[name] = t

            vT2_f16 = vtpool.tile([P, L], F16)
            for name, dst in (("q", qT2), ("k", kT2), ("v", vT2_f16)):
                ps = ppool.tile([P, L], F32, tag="projps")
                for nc_i in range(4):
                    nsl = slice(512 * nc_i, 512 * (nc_i + 1))
                    for kc in range(4):
                        nc.tensor.matmul(
                            ps[:, nsl],
                            w_sb[name][:, P * kc:P * (kc + 1)],
                            xt_sb[kc][:, nsl],
                            start=(kc == 0), stop=(kc == 3),
                        )
                nc.scalar.activation(dst[:], ps[:], Copy)

            # transpose vT2 [i, n] -> v2 chunks [n, i], batched evacuation
            for g in range(0, NT, 4):
                pt = tpool.tile([P, 4 * P], F16, tag="vtr", name=f"vtr{g}")
                for c in range(g, g + 4):
                    nc.tensor.transpose(
                        pt[:, P * (c - g):P * (c - g + 1)],
                        vT2_f16[:, P * c:P * (c + 1)], identh_sb[:])
                nc.vector.tensor_copy(v2[:, P * g:P * (g + 4)], pt[:])

        # ---------------- attention (per head) ----------------
        with ExitStack() as actx:
            spool = actx.enter_context(tc.tile_pool(name="spsum", bufs=2, space="PSUM"))
            opool = actx.enter_context(tc.tile_pool(name="opsum", bufs=1, space="PSUM"))
            tpsum = actx.enter_context(tc.tile_pool(name="tpsum", bufs=2, space="PSUM"))
            apool = actx.enter_context(tc.tile_pool(name="attn", bufs=1))
            atpool = actx.enter_context(tc.tile_pool(name="attnT", bufs=1))
            cpool = actx.enter_context(tc.tile_pool(name="cands", bufs=1))
            smpool = actx.enter_context(tc.tile_pool(name="smalls", bufs=2))

            for h in range(2):
                hsl = slice(HD * h, HD * (h + 1))

                cand = cpool.tile([P, NT * NC8], F32, tag="cand")
                sort16 = cpool.tile([P, NT * NCAND], F32, tag="sort16")
                mrow = cpool.tile([P, NT], F32, tag="mrow")
                bias1 = cpool.tile([P, NT], F32, tag="bias1")
                # attn tiles (fp16), also used as t (shifted scores)
                attn_t = [apool.tile([P, P * (qi + 1)], F16, tag=f"at{qi}", name=f"at{h}_{qi}")
                          for qi in range(NT)]
                attnT_t = [atpool.tile([P, P * (qi + 1)], F16, tag=f"aT{qi}", name=f"aT{h}_{qi}")
                           for qi in range(NT)]

                for qi in range(NT):
                    W = P * (qi + 1)
                    qsl = slice(P * qi, P * (qi + 1))
                    csl = lambda s: slice(qi * NC8 + 8 * s, qi * NC8 + 8 * (s + 1))
                    for half in range(2):
                        hw = W // 2
                        # overlap of diag block [W-P, W) with this half, in
                        # half-local coords
                        mlo = max(0, (W - P) - half * hw)
                        mhi = min(hw, W - half * hw)
                        has_mask = mhi > mlo
                        ps = spool.tile([P, 1024], F32, tag="sps")
                        nchunks = (hw + 511) // 512
                        for ncx in range(nchunks):
                            n0 = 512 * ncx
                            n1 = min(hw, n0 + 512)
                            nc.tensor.matmul(
                                ps[:, n0:n1],
                                qT2[hsl, qsl],
                                kT2[hsl, half * hw + n0:half * hw + n1],
                                start=True, stop=True,
                            )
                        if has_mask:
                            # additive causal mask on (part of) the diag block
                            dlo = mlo + half * hw - (W - P)
                            dhi = mhi + half * hw - (W - P)
                            nc.tensor.matmul(
                                ps[:, mlo:mhi],
                                identh_sb[:],
                                dmask_sb[:, dlo:dhi],
                                start=False, stop=True,
                                skip_group_check=True,
                            )
                        # candidates: top-8 of each quarter (2 per half)
                        for s in range(2):
                            seg = s + 2 * half
                            nc.vector.max(
                                out=cand[:, csl(seg)],
                                in_=ps[:, s * (hw // 2):(s + 1) * (hw // 2)],
                            )
                        if half == 0:
                            ps0 = ps
                        else:
                            # row shift m = max of the 4 segment heads
                            nc.vector.tensor_reduce(
                                mrow[:, qi:qi + 1],
                                cand[:, qi * NC8:(qi + 1) * NC8].rearrange(
                                    "p (s e) -> p s e", e=8)[:, :, 0:1],
                                axis=mybir.AxisListType.XY, op=Alu.max,
                            )
                            # bias1 = 1 - m
                            nc.vector.tensor_scalar(
                                out=bias1[:, qi:qi + 1], in0=mrow[:, qi:qi + 1],
                                scalar1=-1.0, scalar2=1.0,
                                op0=Alu.mult, op1=Alu.add,
                            )
                            # evacuate both halves: t = relu(z - (m-1)) -> fp16
                            nc.scalar.activation(
                                attn_t[qi][:, 0:hw], ps0[:, 0:hw], Relu,
                                bias=bias1[:, qi:qi + 1],
                            )
                            nc.scalar.activation(
                                attn_t[qi][:, hw:W], ps[:, 0:hw], Relu,
                                bias=bias1[:, qi:qi + 1],
                            )

                # sorted top-16 of the 32 candidates, per tile
                for qi in range(NT):
                    c32 = cand[:, qi * NC8:(qi + 1) * NC8]
                    s16 = sort16[:, qi * NCAND:(qi + 1) * NCAND]
                    scr = smpool.tile([P, NC8], F32, tag="scr")
                    nc.vector.max(out=s16[:, 0:8], in_=c32)
                    nc.vector.match_replace(
                        out=scr[:], in_to_replace=s16[:, 0:8], in_values=c32,
                        imm_value=-1e9,
                    )
                    nc.vector.max(out=s16[:, 8:16], in_=scr[:])

                # stacked tau computation (fp32, exact): view [P, NT, NCAND]
                s3 = sort16.rearrange("p (t c) -> p t c", c=NCAND)
                cum = smpool.tile([P, NT * NCAND], F32, tag="cum")
                cum3 = cum.rearrange("p (t c) -> p t c", c=NCAND)
                nc.vector.tensor_copy(cum[:], sort16[:])
                tmp = smpool.tile([P, NT * NCAND], F32, tag="tmp")
                tmp3 = tmp.rearrange("p (t c) -> p t c", c=NCAND)
                src, dst = cum3, tmp3
                srcf, dstf = cum, tmp
                for d in (1, 2, 4, 8):
                    nc.vector.tensor_tensor(
                        out=dst[:, :, d:], in0=src[:, :, d:], in1=src[:, :, :NCAND - d],
                        op=Alu.add,
                    )
                    nc.vector.tensor_copy(dst[:, :, 0:d], src[:, :, 0:d])
                    src, dst = dst, src
                    srcf, dstf = dstf, srcf
                # src now holds cumsum
                # cond = (1 + k*v - S) > 0
                u = smpool.tile([P, NT * NCAND], F32, tag="u")
                nc.vector.tensor_tensor(out=u[:], in0=sort16[:], in1=kvec_sb[:], op=Alu.mult)
                nc.vector.tensor_tensor(out=u[:], in0=u[:], in1=srcf[:], op=Alu.subtract)
                cnd = smpool.tile([P, NT * NCAND], F32, tag="cnd")
                nc.vector.tensor_scalar(
                    out=cnd[:], in0=u[:], scalar1=-1.0, scalar2=None, op0=Alu.is_gt,
                )
                # S_kz = sum(cnd * v); kz = sum(cnd)
                pv = smpool.tile([P, NT * NCAND], F32, tag="pv")
                nc.vector.tensor_tensor(out=pv[:], in0=cnd[:], in1=sort16[:], op=Alu.mult)
                skz = smpool.tile([P, NT], F32, tag="skz")
                nc.vector.tensor_reduce(
                    skz[:], pv.rearrange("p (t c) -> p t c", c=NCAND),
                    axis=mybir.AxisListType.X, op=Alu.add,
                )
                kz = smpool.tile([P, NT], F32, tag="kz")
                nc.vector.tensor_reduce(
                    kz[:], cnd.rearrange("p (t c) -> p t c", c=NCAND),
                    axis=mybir.AxisListType.X, op=Alu.add,
                )
                rkz = smpool.tile([P, NT], F32, tag="rkz")
                nc.vector.reciprocal(rkz[:], kz[:])
                # delta = (S-1)/kz - (m-1) ; store negdelta = -delta
                nc.vector.tensor_scalar(
                    out=skz[:], in0=skz[:], scalar1=-1.0, scalar2=None, op0=Alu.add,
                )
                tauz = smpool.tile([P, NT], F32, tag="tauz")
                nc.vector.tensor_tensor(out=tauz[:], in0=skz[:], in1=rkz[:], op=Alu.mult)
                ndelta = smpool.tile([P, NT], F32, tag="ndelta")
                # ndelta = (m - 1) - tau = -(bias1) ... bias1 = 1-m so m-1 = -bias1
                nc.vector.tensor_tensor(out=ndelta[:], in0=bias1[:], in1=tauz[:], op=Alu.add)
                nc.vector.tensor_scalar(
                    out=ndelta[:], in0=ndelta[:], scalar1=-1.0, scalar2=None, op0=Alu.mult,
                )

                # attn = relu(t - delta), PE-transpose blocks, attn.T @ v
                for ohalf in range(1, -1, -1):
                    psum_o = opool.tile([HD, L // 2], F32, tag="po",
                                        name=f"po{h}_{ohalf}")
                    for qi in range(8 * ohalf + 7, 8 * ohalf - 1, -1):
                        nc.vector.tensor_scalar(
                            out=attn_t[qi][:], in0=attn_t[qi][:],
                            scalar1=ndelta[:, qi:qi + 1], scalar2=0.0,
                            op0=Alu.add, op1=Alu.max,
                        )
                        for g in range(0, qi + 1, 4):
                            gend = min(qi + 1, g + 4)
                            gw = P * (gend - g)
                            pt = tpsum.tile([P, 4 * P], F16, tag="atr",
                                            name=f"atr{h}_{qi}_{g}")
                            for c in range(g, gend):
                                nc.tensor.transpose(
                                    pt[:, P * (c - g):P * (c - g + 1)],
                                    attn_t[qi][:, P * c:P * (c + 1)],
                                    identh_sb[:])
                            nc.vector.tensor_copy(
                                attnT_t[qi][:, P * g:P * g + gw], pt[:, :gw])
                            for c in range(g, gend):
                                nc.tensor.matmul(
                                    psum_o[:, P * (qi - 8 * ohalf):P * (qi - 8 * ohalf + 1)],
                                    v2[:, P * c:P * (c + 1)][:, hsl],
                                    attnT_t[qi][:, P * c:P * (c + 1)],
                                    start=(c == 0), stop=(c == qi),
                                )
                    nc.scalar.activation(
                        hoT2[HD * h:HD * (h + 1),
                             (L // 2) * ohalf:(L // 2) * (ohalf + 1)],
                        psum_o[:], Copy)

        # single fp16 output DMA [128, 2048]
        nc.sync.dma_start(ho, hoT2[:])


_NC_CACHE = {}


def _get_state():
    if "state" in _NC_CACHE:
        return _NC_CACHE["state"]

    import jax
    import jax.numpy as jnp
    from jax.sharding import Mesh, PartitionSpec

    import warnings
    with warnings.catch_warnings():
        warnings.simplefilter("ignore", DeprecationWarning)
        from jax.experimental.shard_map import shard_map

    nc = _build_program()
    b2j.install_neuronx_cc_hook()

    devices = jax.devices()[:8]
    mesh = Mesh(np.asarray(devices), ("core",))
    PC = PartitionSpec("core")

    # ----- stage 1: unpack + all_gather + per-core slicing, on device -----
    def _stage1(p_local):
        pa = jax.lax.all_gather(p_local[0], "core", axis=0)  # [8, NPK] f16
        cid = jax.lax.axis_index("core")
        bb = cid // 4
        j2 = cid % 4
        xall = pa[:, :XSZ].reshape(8, 512, 512)
        xb = jax.lax.dynamic_slice(xall, (4 * bb, 0, 0), (4, 512, 512))
        xt = xb.reshape(L, D).T                            # [512, 2048] f16
        ws = []
        for wi in range(3):
            wf = pa[:, XSZ + wi * WSZ:XSZ + (wi + 1) * WSZ].reshape(D, D)
            wsl = jax.lax.dynamic_slice(wf, (P * j2, 0), (P, D))
            ws.append(wsl.T)                               # [512, 128] f16
        row = jax.lax.broadcasted_iota(jnp.int32, (P, P), 0)
        col = jax.lax.broadcasted_iota(jnp.int32, (P, P), 1)
        dmask = jnp.where(col > row, jnp.float16(NEG), jnp.float16(0.0))
        identh = jnp.eye(P, dtype=jnp.float16)
        kvec = jnp.broadcast_to(
            jnp.tile(jnp.arange(1, NCAND + 1, dtype=jnp.float32), NT)[None, :],
            (P, NT * NCAND))
        zho = jnp.zeros((P, L), jnp.float16)
        return xt, ws[0], ws[1], ws[2], dmask, identh, kvec, zho

    stage1 = jax.jit(shard_map(
        _stage1, mesh=mesh, in_specs=(PC,), out_specs=(PC,) * 8, check_rep=False))

    # ----- stage 2: the Bass NEFF as a custom call on device arrays -----
    partition_name = nc.partition_id_tensor.name if nc.partition_id_tensor else None
    in_names, out_names, out_avals = [], [], []
    for alloc in nc.m.functions[0].allocations:
        if not isinstance(alloc, mybir.MemoryLocationSet):
            continue
        name = alloc.memorylocations[0].name
        if alloc.kind == "ExternalInput":
            if name != partition_name:
                in_names.append(name)
        elif alloc.kind == "ExternalOutput":
            out_names.append(name)
            out_avals.append(jax.core.ShapedArray(
                tuple(alloc.tensor_shape), mybir.dt.np(alloc.dtype)))
    n_params = len(in_names)
    n_outs = len(out_names)
    in_names_all = in_names + out_names
    if partition_name is not None:
        in_names_all.append(partition_name)
    donate = tuple(range(n_params, n_params + n_outs))

    def _body(*args):
        operands = list(args)
        if partition_name is not None:
            operands.append(b2j.partition_id_tensor())
        outs = b2j._bass_exec_p.bind(
            *operands,
            out_avals=tuple(out_avals),
            in_names=tuple(in_names_all),
            out_names=tuple(out_names),
            lowering_input_output_aliases=(),
            sim_require_finite=True,
            sim_require_nnan=True,
            nc=nc,
        )
        return tuple(outs)

    stage2 = jax.jit(
        shard_map(_body, mesh=mesh, in_specs=(PC,) * (n_params + n_outs),
                  out_specs=(PC,) * n_outs, check_rep=False),
        donate_argnums=donate, keep_unused=True)

    state = {"nc": nc, "stage1": stage1, "stage2": stage2,
             "in_names": in_names, "out_names": out_names}
    _NC_CACHE["state"] = state
    return state


def _pack_inputs(x, W_q, W_k, W_v):
    packed = np.empty((8, NPK), np.float16)
    packed[:, :XSZ] = x.reshape(8, XSZ).astype(np.float16)
    # 1/sqrt(head_dim) score scale folded into the q projection
    packed[:, XSZ:XSZ + WSZ] = (W_q * np.float32(0.125)).astype(np.float16).reshape(8, WSZ)
    packed[:, XSZ + WSZ:XSZ + 2 * WSZ] = W_k.astype(np.float16).reshape(8, WSZ)
    packed[:, XSZ + 2 * WSZ:] = W_v.astype(np.float16).reshape(8, WSZ)
    return packed


def _project_out(ho_np, W_o):
    # ho_np [1024, 2048] fp16: rows 128c:128c+128 = core c's two heads' out.T
    out = np.empty((2, L, D), np.float32)
    Wo32 = np.asarray(W_o, dtype=np.float32)
    for bb in range(2):
        A = ho_np[D * bb:D * (bb + 1)].astype(np.float32)   # [512, 2048]
        out[bb] = (Wo32 @ A).T
    return out


def _kernel_traced(x, W_q, W_k, W_v, W_o):
    """Debug/profiling path through run_bass_kernel_spmd (KERNEL_TRACE=1)."""
    nc = _get_state()["nc"]
    dmask_np = np.where(
        np.arange(P)[None, :] > np.arange(P)[:, None],
        np.float16(NEG), np.float16(0.0)).astype(np.float16)
    identh_np = np.eye(P, dtype=np.float16)
    kvec_np = np.broadcast_to(
        np.tile(np.arange(1, NCAND + 1, dtype=np.float32), NT)[None, :],
        (P, NT * NCAND)).copy()
    in_maps = []
    for c in range(8):
        bb = c // 4
        hs = slice(P * (c % 4), P * (c % 4 + 1))
        in_maps.append({
            "xt": np.ascontiguousarray(x[bb].T).astype(np.float16),
            "wq": np.ascontiguousarray((W_q[hs] * np.float32(0.125)).T).astype(np.float16),
            "wk": np.ascontiguousarray(W_k[hs].T).astype(np.float16),
            "wv": np.ascontiguousarray(W_v[hs].T).astype(np.float16),
            "dmask": dmask_np, "identh": identh_np, "kvec": kvec_np,
        })
    res = run_bass_kernel_spmd(nc, in_maps, core_ids=list(range(8)), trace=True)
    _NC_CACHE["last_results"] = res
    ho_np = np.concatenate([res.results[c]["ho"] for c in range(8)], axis=0)
    return _project_out(ho_np, W_o)


def kernel(x, W_q, W_k, W_v, W_o):
    x = np.asarray(x, dtype=np.float32)
    W_q = np.asarray(W_q, dtype=np.float32)
    W_k = np.asarray(W_k, dtype=np.float32)
    W_v = np.asarray(W_v, dtype=np.float32)
    W_o = np.asarray(W_o, dtype=np.float32)

    if bool(int(os.environ.get("KERNEL_TRACE", "0"))):
        return _kernel_traced(x, W_q, W_k, W_v, W_o)

    st = _get_state()
    packed = _pack_inputs(x, W_q, W_k, W_v)
    s1 = st["stage1"](packed)
    ho = st["stage2"](*s1)[0]
    ho_np = np.asarray(ho)  # [1024, 2048] fp16
    return _project_out(ho_np, W_o)


# revision 5
# speedup vs baseline: 15.4536x; 1.0458x over previous
"""Trainium2 Bass kernel for CustomSimplexMappingAttention (sparsemax attention).

Sharding: batch*head parallel across 8 cores. Core c handles batch b=c//4 and
heads {2*(c%4), 2*(c%4)+1}. Each core computes its two heads' attention output
(pre output-projection); the host applies W_o.

Wall-clock is dominated by the axon host<->device tunnel, so the call is
structured to minimize transferred bytes and round trips:
  - one packed fp16 upload [8, 360448] (x sliced by seq + W_q/k/v sliced by row;
    every byte uploaded exactly once),
  - stage-1 XLA jit on the 8 cores: all_gather + per-core slicing/transposes,
    constants (causal mask, identity, k-vector) and the zero donation buffers
    generated on device,
  - stage-2: the Bass NEFF via the _bass_exec custom call on device-resident
    arrays (jit cached across calls),
  - one fp16 download [1024, 2048] of per-head outputs; host does the W_o
    projection (two ~0.5 GFLOP sgemms).

Per-core Bass algorithm (per head):
  scores tile [128 q, W] (causal W=128*(qi+1)) in PSUM via fp16 matmuls
  candidate extraction: top-8 of each quarter-segment via DVE max8 (exact fp32)
  per-tile shift m (max of cands); ACT relu(z-(m-1)) evacuates PSUM -> fp16 t
  sorted top-16 per row (max8+match_replace on the 32 candidates), cumsum,
  closed-form sparsemax threshold tau (all fp32, exact)
  attn = relu(t - (tau-m+1)) fp16 (DVE), PE-transposed into k-major layout,
  fp16 matmuls attn.T @ v accumulate out.T -> hoT2 fp16 -> single DMA out.
"""

import os
from contextlib import ExitStack

import numpy as np

import concourse.bass as bass
import concourse.tile as tile
from concourse import bacc
from concourse import mybir
from concourse import bass2jax as b2j
from concourse.bass_utils import run_bass_kernel_spmd

F32 = mybir.dt.float32
F16 = mybir.dt.float16

P = 128
L = 2048
D = 512
HD = 64
NT = L // P  # 16 q tiles
NEG = -60000.0  # fp16-representable stand-in for -inf in the causal mask
NSEG = 4   # candidate segments per row
NC8 = 8 * NSEG  # 32 raw candidates
NCAND = 16  # sorted candidates kept

XSZ = 512 * 512          # packed per-core x slab elements
WSZ = 64 * 512           # packed per-core weight slab elements (per weight)
NPK = XSZ + 3 * WSZ      # packed per-core upload row


def _build_program():
    nc = bacc.Bacc("TRN2", target_bir_lowering=False, debug=False)

    xt = nc.dram_tensor("xt", [D, L], F16, kind="ExternalInput").ap()
    wq = nc.dram_tensor("wq", [D, P], F16, kind="ExternalInput").ap()
    wk = nc.dram_tensor("wk", [D, P], F16, kind="ExternalInput").ap()
    wv = nc.dram_tensor("wv", [D, P], F16, kind="ExternalInput").ap()
    dmask = nc.dram_tensor("dmask", [P, P], F16, kind="ExternalInput").ap()
    identh = nc.dram_tensor("identh", [P, P], F16, kind="ExternalInput").ap()
    kvec = nc.dram_tensor("kvec", [P, NT * NCAND], F32, kind="ExternalInput").ap()
    ho = nc.dram_tensor("ho", [P, L], F16, kind="ExternalOutput").ap()
    with tile.TileContext(nc) as tc:
        _kernel_body(tc, ho, xt, wq, wk, wv, dmask, identh, kvec)
    nc.finalize()
    return nc


def _kernel_body(tc, ho, xt, wq, wk, wv, dmask, identh, kvec):
    nc = tc.nc
    Relu = mybir.ActivationFunctionType.Relu
    Copy = mybir.ActivationFunctionType.Copy
    Alu = mybir.AluOpType

    with ExitStack() as ctx:
        consts = ctx.enter_context(tc.tile_pool(name="consts", bufs=1))
        dmask_sb = consts.tile([P, P], F16)
        nc.sync.dma_start(dmask_sb[:], dmask)
        identh_sb = consts.tile([P, P], F16)
        nc.sync.dma_start(identh_sb[:], identh)
        kvec_sb = consts.tile([P, NT * NCAND], F32)
        nc.sync.dma_start(kvec_sb[:], kvec)

        # persistent activation tiles
        persist = ctx.enter_context(tc.tile_pool(name="persist", bufs=1))
        qT2 = persist.tile([P, L], F16)    # q.T both heads [i(2h), l]
        kT2 = persist.tile([P, L], F16)
        v2 = persist.tile([P, L], F16)     # v chunks: block c cols -> [n in c, i(2h)]
        hoT2 = persist.tile([P, L], F16)   # head outs .T, head h rows 64h:64h+64

        # ---------------- projections ----------------
        with ExitStack() as pctx:
            xpool = pctx.enter_context(tc.tile_pool(name="xt", bufs=1))
            wpool = pctx.enter_context(tc.tile_pool(name="w", bufs=1))
            ppool = pctx.enter_context(tc.tile_pool(name="pproj", bufs=1, space="PSUM"))
            tpool = pctx.enter_context(tc.tile_pool(name="ptrans", bufs=4, space="PSUM"))
            vtpool = pctx.enter_context(tc.tile_pool(name="vt", bufs=1))

            xt_sb = [xpool.tile([P, L], F16, tag=f"xt{i}", name=f"xt{i}") for i in range(4)]
            for kc in range(4):
                nc.sync.dma_start(xt_sb[kc][:], xt[P * kc:P * (kc + 1), :])
            w_sb = {}
            for name, w in (("q", wq), ("k", wk), ("v", wv)):
                t = wpool.tile([P, 4 * P], F16, tag=f"w{name}")
                nc.sync.dma_start(
                    t.rearrange("p (c i) -> p c i", i=P),
                    w.rearrange("(c p) i -> p c i", p=P),
                )
                w_sb[name] = t

            vT2_f16 = vtpool.tile([P, L], F16)
            for name, dst in (("q", qT2), ("k", kT2), ("v", vT2_f16)):
                ps = ppool.tile([P, L], F32, tag="projps")
                for nc_i in range(4):
                    nsl = slice(512 * nc_i, 512 * (nc_i + 1))
                    for kc in range(4):
                        nc.tensor.matmul(
                            ps[:, nsl],
                            w_sb[name][:, P * kc:P * (kc + 1)],
                            xt_sb[kc][:, nsl],
                            start=(kc == 0), stop=(kc == 3),
                        )
                nc.scalar.activation(dst[:], ps[:], Copy)

            # transpose vT2 [i, n] -> v2 chunks [n, i], batched evacuation
            for g in range(0, NT, 4):
                pt = tpool.tile([P, 4 * P], F16, tag="vtr", name=f"vtr{g}")
                for c in range(g, g + 4):
                    nc.tensor.transpose(
                        pt[:, P * (c - g):P * (c - g + 1)],
                        vT2_f16[:, P * c:P * (c + 1)], identh_sb[:])
                nc.vector.tensor_copy(v2[:, P * g:P * (g + 4)], pt[:])

        # ---------------- attention (per head) ----------------
        with ExitStack() as actx:
            spool = actx.enter_context(tc.tile_pool(name="spsum", bufs=2, space="PSUM"))
            opool = actx.enter_context(tc.tile_pool(name="opsum", bufs=1, space="PSUM"))
            tpsum = actx.enter_context(tc.tile_pool(name="tpsum", bufs=2, space="PSUM"))
            apool = actx.enter_context(tc.tile_pool(name="attn", bufs=1))
            atpool = actx.enter_context(tc.tile_pool(name="attnT", bufs=1))
            cpool = actx.enter_context(tc.tile_pool(name="cands", bufs=1))
            smpool = actx.enter_context(tc.tile_pool(name="smalls", bufs=2))

            for h in range(2):
                hsl = slice(HD * h, HD * (h + 1))

                cand = cpool.tile([P, NT * NC8], F32, tag="cand")
                sort16 = cpool.tile([P, NT * NCAND], F32, tag="sort16")
                mrow = cpool.tile([P, NT], F32, tag="mrow")
                bias1 = cpool.tile([P, NT], F32, tag="bias1")
                # attn tiles (fp16), also used as t (shifted scores)
                attn_t = [apool.tile([P, P * (qi + 1)], F16, tag=f"at{qi}", name=f"at{h}_{qi}")
                          for qi in range(NT)]
                attnT_t = [atpool.tile([P, P * (qi + 1)], F16, tag=f"aT{qi}", name=f"aT{h}_{qi}")
                           for qi in range(NT)]

                for qi in range(NT):
                    W = P * (qi + 1)
                    qsl = slice(P * qi, P * (qi + 1))
                    csl = lambda s: slice(qi * NC8 + 8 * s, qi * NC8 + 8 * (s + 1))
                    for half in range(2):
                        hw = W // 2
                        # overlap of diag block [W-P, W) with this half, in
                        # half-local coords
                        mlo = max(0, (W - P) - half * hw)
                        mhi = min(hw, W - half * hw)
                        has_mask = mhi > mlo
                        ps = spool.tile([P, 1024], F32, tag="sps")
                        nchunks = (hw + 511) // 512
                        for ncx in range(nchunks):
                            n0 = 512 * ncx
                            n1 = min(hw, n0 + 512)
                            nc.tensor.matmul(
                                ps[:, n0:n1],
                                qT2[hsl, qsl],
                                kT2[hsl, half * hw + n0:half * hw + n1],
                                start=True, stop=True,
                            )
                        if has_mask:
                            # additive causal mask on (part of) the diag block
                            dlo = mlo + half * hw - (W - P)
                            dhi = mhi + half * hw - (W - P)
                            nc.tensor.matmul(
                                ps[:, mlo:mhi],
                                identh_sb[:],
                                dmask_sb[:, dlo:dhi],
                                start=False, stop=True,
                                skip_group_check=True,
                            )
                        # candidates: top-8 of each quarter (2 per half)
                        for s in range(2):
                            seg = s + 2 * half
                            nc.vector.max(
                                out=cand[:, csl(seg)],
                                in_=ps[:, s * (hw // 2):(s + 1) * (hw // 2)],
                            )
                        if half == 0:
                            ps0 = ps
                        else:
                            # row shift m = max of the 4 segment heads
                            nc.vector.tensor_reduce(
                                mrow[:, qi:qi + 1],
                                cand[:, qi * NC8:(qi + 1) * NC8].rearrange(
                                    "p (s e) -> p s e", e=8)[:, :, 0:1],
                                axis=mybir.AxisListType.XY, op=Alu.max,
                            )
                            # bias1 = 1 - m
                            nc.vector.tensor_scalar(
                                out=bias1[:, qi:qi + 1], in0=mrow[:, qi:qi + 1],
                                scalar1=-1.0, scalar2=1.0,
                                op0=Alu.mult, op1=Alu.add,
                            )
                            # evacuate both halves: t = relu(z - (m-1)) -> fp16
                            nc.scalar.activation(
                                attn_t[qi][:, 0:hw], ps0[:, 0:hw], Relu,
                                bias=bias1[:, qi:qi + 1],
                            )
                            nc.scalar.activation(
                                attn_t[qi][:, hw:W], ps[:, 0:hw], Relu,
                                bias=bias1[:, qi:qi + 1],
                            )

                # sorted top-16 of the 32 candidates, per tile
                for qi in range(NT):
                    c32 = cand[:, qi * NC8:(qi + 1) * NC8]
                    s16 = sort16[:, qi * NCAND:(qi + 1) * NCAND]
                    scr = smpool.tile([P, NC8], F32, tag="scr")
                    nc.vector.max(out=s16[:, 0:8], in_=c32)
                    nc.vector.match_replace(
                        out=scr[:], in_to_replace=s16[:, 0:8], in_values=c32,
                        imm_value=-1e9,
                    )
                    nc.vector.max(out=s16[:, 8:16], in_=scr[:])

                # stacked tau computation (fp32, exact): view [P, NT, NCAND]
                s3 = sort16.rearrange("p (t c) -> p t c", c=NCAND)
                cum = smpool.tile([P, NT * NCAND], F32, tag="cum")
                cum3 = cum.rearrange("p (t c) -> p t c", c=NCAND)
                nc.vector.tensor_copy(cum[:], sort16[:])
                tmp = smpool.tile([P, NT * NCAND], F32, tag="tmp")
                tmp3 = tmp.rearrange("p (t c) -> p t c", c=NCAND)
                src, dst = cum3, tmp3
                srcf, dstf = cum, tmp
                for d in (1, 2, 4, 8):
                    nc.vector.tensor_tensor(
                        out=dst[:, :, d:], in0=src[:, :, d:], in1=src[:, :, :NCAND - d],
                        op=Alu.add,
                    )
                    nc.vector.tensor_copy(dst[:, :, 0:d], src[:, :, 0:d])
                    src, dst = dst, src
                    srcf, dstf = dstf, srcf
                # src now holds cumsum
                # cond = (1 + k*v - S) > 0
                u = smpool.tile([P, NT * NCAND], F32, tag="u")
                nc.vector.tensor_tensor(out=u[:], in0=sort16[:], in1=kvec_sb[:], op=Alu.mult)
                nc.vector.tensor_tensor(out=u[:], in0=u[:], in1=srcf[:], op=Alu.subtract)
                cnd = smpool.tile([P, NT * NCAND], F32, tag="cnd")
                nc.vector.tensor_scalar(
                    out=cnd[:], in0=u[:], scalar1=-1.0, scalar2=None, op0=Alu.is_gt,
                )
                # S_kz = sum(cnd * v); kz = sum(cnd)
                pv = smpool.tile([P, NT * NCAND], F32, tag="pv")
                nc.vector.tensor_tensor(out=pv[:], in0=cnd[:], in1=sort16[:], op=Alu.mult)
                skz = smpool.tile([P, NT], F32, tag="skz")
                nc.vector.tensor_reduce(
                    skz[:], pv.rearrange("p (t c) -> p t c", c=NCAND),
                    axis=mybir.AxisListType.X, op=Alu.add,
                )
                kz = smpool.tile([P, NT], F32, tag="kz")
                nc.vector.tensor_reduce(
                    kz[:], cnd.rearrange("p (t c) -> p t c", c=NCAND),
                    axis=mybir.AxisListType.X, op=Alu.add,
                )
                rkz = smpool.tile([P, NT], F32, tag="rkz")
                nc.vector.reciprocal(rkz[:], kz[:])
                # delta = (S-1)/kz - (m-1) ; store negdelta = -delta
                nc.vector.tensor_scalar(
                    out=skz[:], in0=skz[:], scalar1=-1.0, scalar2=None, op0=Alu.add,
                )
                tauz = smpool.tile([P, NT], F32, tag="tauz")
                nc.vector.tensor_tensor(out=tauz[:], in0=skz[:], in1=rkz[:], op=Alu.mult)
                ndelta = smpool.tile([P, NT], F32, tag="ndelta")
                # ndelta = (m - 1) - tau = -(bias1) ... bias1 = 1-m so m-1 = -bias1
                nc.vector.tensor_tensor(out=ndelta[:], in0=bias1[:], in1=tauz[:], op=Alu.add)
                nc.vector.tensor_scalar(
                    out=ndelta[:], in0=ndelta[:], scalar1=-1.0, scalar2=None, op0=Alu.mult,
                )

                # attn = relu(t - delta), PE-transpose blocks, attn.T @ v
                for ohalf in range(1, -1, -1):
                    psum_o = opool.tile([HD, L // 2], F32, tag="po",
                                        name=f"po{h}_{ohalf}")
                    for qi in range(8 * ohalf + 7, 8 * ohalf - 1, -1):
                        nc.vector.tensor_scalar(
                            out=attn_t[qi][:], in0=attn_t[qi][:],
                            scalar1=ndelta[:, qi:qi + 1], scalar2=0.0,
                            op0=Alu.add, op1=Alu.max,
                        )
                        for g in range(0, qi + 1, 4):
                            gend = min(qi + 1, g + 4)
                            gw = P * (gend - g)
                            pt = tpsum.tile([P, 4 * P], F16, tag="atr",
                                            name=f"atr{h}_{qi}_{g}")
                            for c in range(g, gend):
                                nc.tensor.transpose(
                                    pt[:, P * (c - g):P * (c - g + 1)],
                                    attn_t[qi][:, P * c:P * (c + 1)],
                                    identh_sb[:])
                            nc.vector.tensor_copy(
                                attnT_t[qi][:, P * g:P * g + gw], pt[:, :gw])
                            for c in range(g, gend):
                                nc.tensor.matmul(
                                    psum_o[:, P * (qi - 8 * ohalf):P * (qi - 8 * ohalf + 1)],
                                    v2[:, P * c:P * (c + 1)][:, hsl],
                                    attnT_t[qi][:, P * c:P * (c + 1)],
                                    start=(c == 0), stop=(c == qi),
                                )
                    nc.scalar.activation(
                        hoT2[HD * h:HD * (h + 1),
                             (L // 2) * ohalf:(L // 2) * (ohalf + 1)],
                        psum_o[:], Copy)

        # single fp16 output DMA [128, 2048]
        nc.sync.dma_start(ho, hoT2[:])


_NC_CACHE = {}


def _get_state():
    if "state" in _NC_CACHE:
        return _NC_CACHE["state"]

    import jax
    import jax.numpy as jnp
    from jax.sharding import Mesh, PartitionSpec

    import warnings
    with warnings.catch_warnings():
        warnings.simplefilter("ignore", DeprecationWarning)
        from jax.experimental.shard_map import shard_map

    nc = _build_program()
    b2j.install_neuronx_cc_hook()

    devices = jax.devices()[:8]
    mesh = Mesh(np.asarray(devices), ("core",))
    PC = PartitionSpec("core")

    # ----- stage 1: unpack + all_gather + per-core slicing, on device -----
    def _stage1(p_local):
        pa = jax.lax.all_gather(p_local[0], "core", axis=0)  # [8, NPK] f16
        cid = jax.lax.axis_index("core")
        bb = cid // 4
        j2 = cid % 4
        xall = pa[:, :XSZ].reshape(8, 512, 512)
        xb = jax.lax.dynamic_slice(xall, (4 * bb, 0, 0), (4, 512, 512))
        xt = xb.reshape(L, D).T                            # [512, 2048] f16
        ws = []
        for wi in range(3):
            wf = pa[:, XSZ + wi * WSZ:XSZ + (wi + 1) * WSZ].reshape(D, D)
            wsl = jax.lax.dynamic_slice(wf, (P * j2, 0), (P, D))
            ws.append(wsl.T)                               # [512, 128] f16
        row = jax.lax.broadcasted_iota(jnp.int32, (P, P), 0)
        col = jax.lax.broadcasted_iota(jnp.int32, (P, P), 1)
        dmask = jnp.where(col > row, jnp.float16(NEG), jnp.float16(0.0))
        identh = jnp.eye(P, dtype=jnp.float16)
        kvec = jnp.broadcast_to(
            jnp.tile(jnp.arange(1, NCAND + 1, dtype=jnp.float32), NT)[None, :],
            (P, NT * NCAND))
        zho = jnp.zeros((P, L), jnp.float16)
        return xt, ws[0], ws[1], ws[2], dmask, identh, kvec, zho

    stage1 = jax.jit(shard_map(
        _stage1, mesh=mesh, in_specs=(PC,), out_specs=(PC,) * 8, check_rep=False))

    # ----- stage 2: the Bass NEFF as a custom call on device arrays -----
    partition_name = nc.partition_id_tensor.name if nc.partition_id_tensor else None
    in_names, out_names, out_avals = [], [], []
    for alloc in nc.m.functions[0].allocations:
        if not isinstance(alloc, mybir.MemoryLocationSet):
            continue
        name = alloc.memorylocations[0].name
        if alloc.kind == "ExternalInput":
            if name != partition_name:
                in_names.append(name)
        elif alloc.kind == "ExternalOutput":
            out_names.append(name)
            out_avals.append(jax.core.ShapedArray(
                tuple(alloc.tensor_shape), mybir.dt.np(alloc.dtype)))
    n_params = len(in_names)
    n_outs = len(out_names)
    in_names_all = in_names + out_names
    if partition_name is not None:
        in_names_all.append(partition_name)
    donate = tuple(range(n_params, n_params + n_outs))

    def _body(*args):
        operands = list(args)
        if partition_name is not None:
            operands.append(b2j.partition_id_tensor())
        outs = b2j._bass_exec_p.bind(
            *operands,
            out_avals=tuple(out_avals),
            in_names=tuple(in_names_all),
            out_names=tuple(out_names),
            lowering_input_output_aliases=(),
            sim_require_finite=True,
            sim_require_nnan=True,
            nc=nc,
        )
        return tuple(outs)

    stage2 = jax.jit(
        shard_map(_body, mesh=mesh, in_specs=(PC,) * (n_params + n_outs),
                  out_specs=(PC,) * n_outs, check_rep=False),
        donate_argnums=donate, keep_unused=True)

    state = {"nc": nc, "stage1": stage1, "stage2": stage2,
             "in_names": in_names, "out_names": out_names}
    _NC_CACHE["state"] = state
    return state


def _pack_inputs(x, W_q, W_k, W_v):
    packed = np.empty((8, NPK), np.float16)
    packed[:, :XSZ] = x.reshape(8, XSZ).astype(np.float16)
    # 1/sqrt(head_dim) score scale folded into the q projection
    packed[:, XSZ:XSZ + WSZ] = (W_q * np.float32(0.125)).astype(np.float16).reshape(8, WSZ)
    packed[:, XSZ + WSZ:XSZ + 2 * WSZ] = W_k.astype(np.float16).reshape(8, WSZ)
    packed[:, XSZ + 2 * WSZ:] = W_v.astype(np.float16).reshape(8, WSZ)
    return packed


def _project_out(ho_np, W_o):
    # ho_np [1024, 2048] fp16: rows 128c:128c+128 = core c's two heads' out.T
    from concurrent.futures import ThreadPoolExecutor
    out = np.empty((2, L, D), np.float32)
    Wo32 = np.asarray(W_o, dtype=np.float32)

    def _one(bb):
        A = ho_np[D * bb:D * (bb + 1)].astype(np.float32)   # [512, 2048]
        out[bb] = (Wo32 @ A).T

    with ThreadPoolExecutor(2) as ex:
        list(ex.map(_one, range(2)))
    return out


def _kernel_traced(x, W_q, W_k, W_v, W_o):
    """Debug/profiling path through run_bass_kernel_spmd (KERNEL_TRACE=1)."""
    nc = _get_state()["nc"]
    dmask_np = np.where(
        np.arange(P)[None, :] > np.arange(P)[:, None],
        np.float16(NEG), np.float16(0.0)).astype(np.float16)
    identh_np = np.eye(P, dtype=np.float16)
    kvec_np = np.broadcast_to(
        np.tile(np.arange(1, NCAND + 1, dtype=np.float32), NT)[None, :],
        (P, NT * NCAND)).copy()
    in_maps = []
    for c in range(8):
        bb = c // 4
        hs = slice(P * (c % 4), P * (c % 4 + 1))
        in_maps.append({
            "xt": np.ascontiguousarray(x[bb].T).astype(np.float16),
            "wq": np.ascontiguousarray((W_q[hs] * np.float32(0.125)).T).astype(np.float16),
            "wk": np.ascontiguousarray(W_k[hs].T).astype(np.float16),
            "wv": np.ascontiguousarray(W_v[hs].T).astype(np.float16),
            "dmask": dmask_np, "identh": identh_np, "kvec": kvec_np,
        })
    try:
        res = run_bass_kernel_spmd(nc, in_maps, core_ids=list(range(8)), trace=True)
    except Exception:
        res = run_bass_kernel_spmd(nc, in_maps, core_ids=list(range(8)), trace=False)
    _NC_CACHE["last_results"] = res
    ho_np = np.concatenate([res.results[c]["ho"] for c in range(8)], axis=0)
    return _project_out(ho_np, W_o)


def kernel(x, W_q, W_k, W_v, W_o):
    x = np.asarray(x, dtype=np.float32)
    W_q = np.asarray(W_q, dtype=np.float32)
    W_k = np.asarray(W_k, dtype=np.float32)
    W_v = np.asarray(W_v, dtype=np.float32)
    W_o = np.asarray(W_o, dtype=np.float32)

    if bool(int(os.environ.get("KERNEL_TRACE", "0"))):
        return _kernel_traced(x, W_q, W_k, W_v, W_o)

    st = _get_state()
    packed = _pack_inputs(x, W_q, W_k, W_v)
    s1 = st["stage1"](packed)
    ho = st["stage2"](*s1)[0]
    ho_np = np.asarray(ho)  # [1024, 2048] fp16
    return _project_out(ho_np, W_o)


# revision 6
# speedup vs baseline: 15.5962x; 1.0092x over previous
"""Trainium2 Bass kernel for CustomSimplexMappingAttention (sparsemax attention).

Sharding: batch*head parallel across 8 cores. Core c handles batch b=c//4 and
heads {2*(c%4), 2*(c%4)+1}. Each core computes its two heads' attention output
(pre output-projection); the host applies W_o.

Wall-clock is dominated by the axon host<->device tunnel, so the call is
structured to minimize transferred bytes and round trips:
  - one packed fp16 upload [8, 360448] (x sliced by seq + W_q/k/v sliced by row;
    every byte uploaded exactly once),
  - stage-1 XLA jit on the 8 cores: all_gather + per-core slicing/transposes,
    constants (causal mask, identity, k-vector) and the zero donation buffers
    generated on device,
  - stage-2: the Bass NEFF via the _bass_exec custom call on device-resident
    arrays (jit cached across calls),
  - one fp16 download [1024, 2048] of per-head outputs; host does the W_o
    projection (two ~0.5 GFLOP sgemms).

Per-core Bass algorithm (per head):
  scores tile [128 q, W] (causal W=128*(qi+1)) in PSUM via fp16 matmuls
  candidate extraction: top-8 of each quarter-segment via DVE max8 (exact fp32)
  per-tile shift m (max of cands); ACT relu(z-(m-1)) evacuates PSUM -> fp16 t
  sorted top-16 per row (max8+match_replace on the 32 candidates), cumsum,
  closed-form sparsemax threshold tau (all fp32, exact)
  attn = relu(t - (tau-m+1)) fp16 (DVE), PE-transposed into k-major layout,
  fp16 matmuls attn.T @ v accumulate out.T -> hoT2 fp16 -> single DMA out.
"""

import os
from contextlib import ExitStack

import numpy as np

import concourse.bass as bass
import concourse.tile as tile
from concourse import bacc
from concourse import mybir
from concourse import bass2jax as b2j
from concourse.bass_utils import run_bass_kernel_spmd

F32 = mybir.dt.float32
F16 = mybir.dt.float16

P = 128
L = 2048
D = 512
HD = 64
NT = L // P  # 16 q tiles
NEG = -60000.0  # fp16-representable stand-in for -inf in the causal mask
NSEG = 4   # candidate segments per row
NC8 = 8 * NSEG  # 32 raw candidates
NCAND = 16  # sorted candidates kept

XSZ = 512 * 512          # packed per-core x slab elements
WSZ = 64 * 512           # packed per-core weight slab elements (per weight)
NPK = XSZ + 3 * WSZ      # packed per-core upload row


def _build_program():
    nc = bacc.Bacc("TRN2", target_bir_lowering=False, debug=False)

    xt = nc.dram_tensor("xt", [D, L], F16, kind="ExternalInput").ap()
    wq = nc.dram_tensor("wq", [D, P], F16, kind="ExternalInput").ap()
    wk = nc.dram_tensor("wk", [D, P], F16, kind="ExternalInput").ap()
    wv = nc.dram_tensor("wv", [D, P], F16, kind="ExternalInput").ap()
    dmask = nc.dram_tensor("dmask", [P, P], F16, kind="ExternalInput").ap()
    identh = nc.dram_tensor("identh", [P, P], F16, kind="ExternalInput").ap()
    kvec = nc.dram_tensor("kvec", [P, NT * NCAND], F32, kind="ExternalInput").ap()
    ho = nc.dram_tensor("ho", [P, L], F16, kind="ExternalOutput").ap()
    with tile.TileContext(nc) as tc:
        _kernel_body(tc, ho, xt, wq, wk, wv, dmask, identh, kvec)
    nc.finalize()
    return nc


def _kernel_body(tc, ho, xt, wq, wk, wv, dmask, identh, kvec):
    nc = tc.nc
    Relu = mybir.ActivationFunctionType.Relu
    Copy = mybir.ActivationFunctionType.Copy
    Alu = mybir.AluOpType

    with ExitStack() as ctx:
        consts = ctx.enter_context(tc.tile_pool(name="consts", bufs=1))
        dmask_sb = consts.tile([P, P], F16)
        nc.sync.dma_start(dmask_sb[:], dmask)
        identh_sb = consts.tile([P, P], F16)
        nc.sync.dma_start(identh_sb[:], identh)
        kvec_sb = consts.tile([P, NT * NCAND], F32)
        nc.sync.dma_start(kvec_sb[:], kvec)

        # persistent activation tiles
        persist = ctx.enter_context(tc.tile_pool(name="persist", bufs=1))
        qT2 = persist.tile([P, L], F16)    # q.T both heads [i(2h), l]
        kT2 = persist.tile([P, L], F16)
        v2 = persist.tile([P, L], F16)     # v chunks: block c cols -> [n in c, i(2h)]
        hoT2 = persist.tile([P, L], F16)   # head outs .T, head h rows 64h:64h+64

        # ---------------- projections ----------------
        with ExitStack() as pctx:
            xpool = pctx.enter_context(tc.tile_pool(name="xt", bufs=1))
            wpool = pctx.enter_context(tc.tile_pool(name="w", bufs=1))
            ppool = pctx.enter_context(tc.tile_pool(name="pproj", bufs=1, space="PSUM"))
            tpool = pctx.enter_context(tc.tile_pool(name="ptrans", bufs=4, space="PSUM"))
            vtpool = pctx.enter_context(tc.tile_pool(name="vt", bufs=1))

            xt_sb = [xpool.tile([P, L], F16, tag=f"xt{i}", name=f"xt{i}") for i in range(4)]
            for kc in range(4):
                nc.sync.dma_start(xt_sb[kc][:], xt[P * kc:P * (kc + 1), :])
            w_sb = {}
            for name, w in (("q", wq), ("k", wk), ("v", wv)):
                t = wpool.tile([P, 4 * P], F16, tag=f"w{name}")
                nc.sync.dma_start(
                    t.rearrange("p (c i) -> p c i", i=P),
                    w.rearrange("(c p) i -> p c i", p=P),
                )
                w_sb[name] = t

            vT2_f16 = vtpool.tile([P, L], F16)
            for name, dst in (("q", qT2), ("k", kT2), ("v", vT2_f16)):
                ps = ppool.tile([P, L], F32, tag="projps")
                for nc_i in range(4):
                    nsl = slice(512 * nc_i, 512 * (nc_i + 1))
                    for kc in range(4):
                        nc.tensor.matmul(
                            ps[:, nsl],
                            w_sb[name][:, P * kc:P * (kc + 1)],
                            xt_sb[kc][:, nsl],
                            start=(kc == 0), stop=(kc == 3),
                        )
                nc.scalar.activation(dst[:], ps[:], Copy)

            # transpose vT2 [i, n] -> v2 chunks [n, i], batched evacuation
            for g in range(0, NT, 4):
                pt = tpool.tile([P, 4 * P], F16, tag="vtr", name=f"vtr{g}")
                for c in range(g, g + 4):
                    nc.tensor.transpose(
                        pt[:, P * (c - g):P * (c - g + 1)],
                        vT2_f16[:, P * c:P * (c + 1)], identh_sb[:])
                nc.vector.tensor_copy(v2[:, P * g:P * (g + 4)], pt[:])

        # ---------------- attention (per head) ----------------
        with ExitStack() as actx:
            spool = actx.enter_context(tc.tile_pool(name="spsum", bufs=2, space="PSUM"))
            opool = actx.enter_context(tc.tile_pool(name="opsum", bufs=1, space="PSUM"))
            tpsum = actx.enter_context(tc.tile_pool(name="tpsum", bufs=2, space="PSUM"))
            apool = actx.enter_context(tc.tile_pool(name="attn", bufs=1))
            atpool = actx.enter_context(tc.tile_pool(name="attnT", bufs=1))
            cpool = actx.enter_context(tc.tile_pool(name="cands", bufs=1))
            smpool = actx.enter_context(tc.tile_pool(name="smalls", bufs=2))

            for h in range(2):
                hsl = slice(HD * h, HD * (h + 1))

                cand = cpool.tile([P, NT * NC8], F32, tag="cand")
                sort16 = cpool.tile([P, NT * NCAND], F32, tag="sort16")
                mrow = cpool.tile([P, NT], F32, tag="mrow")
                bias1 = cpool.tile([P, NT], F32, tag="bias1")
                # attn tiles (fp16), also used as t (shifted scores)
                attn_t = [apool.tile([P, P * (qi + 1)], F16, tag=f"at{qi}", name=f"at{h}_{qi}")
                          for qi in range(NT)]
                attnT_t = [atpool.tile([P, P * (qi + 1)], F16, tag=f"aT{qi}", name=f"aT{h}_{qi}")
                           for qi in range(NT)]

                for qi in range(NT):
                    W = P * (qi + 1)
                    qsl = slice(P * qi, P * (qi + 1))
                    csl = lambda s: slice(qi * NC8 + 8 * s, qi * NC8 + 8 * (s + 1))
                    for half in range(2):
                        hw = W // 2
                        # overlap of diag block [W-P, W) with this half, in
                        # half-local coords
                        mlo = max(0, (W - P) - half * hw)
                        mhi = min(hw, W - half * hw)
                        has_mask = mhi > mlo
                        ps = spool.tile([P, 1024], F32, tag="sps")
                        nchunks = (hw + 511) // 512
                        for ncx in range(nchunks):
                            n0 = 512 * ncx
                            n1 = min(hw, n0 + 512)
                            nc.tensor.matmul(
                                ps[:, n0:n1],
                                qT2[hsl, qsl],
                                kT2[hsl, half * hw + n0:half * hw + n1],
                                start=True, stop=True,
                            )
                        if has_mask:
                            # additive causal mask on (part of) the diag block
                            dlo = mlo + half * hw - (W - P)
                            dhi = mhi + half * hw - (W - P)
                            nc.tensor.matmul(
                                ps[:, mlo:mhi],
                                identh_sb[:],
                                dmask_sb[:, dlo:dhi],
                                start=False, stop=True,
                                skip_group_check=True,
                            )
                        # candidates: top-8 of each quarter (2 per half)
                        for s in range(2):
                            seg = s + 2 * half
                            nc.vector.max(
                                out=cand[:, csl(seg)],
                                in_=ps[:, s * (hw // 2):(s + 1) * (hw // 2)],
                            )
                        if half == 0:
                            ps0 = ps
                        else:
                            # row shift m = max of the 4 segment heads
                            nc.vector.tensor_reduce(
                                mrow[:, qi:qi + 1],
                                cand[:, qi * NC8:(qi + 1) * NC8].rearrange(
                                    "p (s e) -> p s e", e=8)[:, :, 0:1],
                                axis=mybir.AxisListType.XY, op=Alu.max,
                            )
                            # bias1 = 1 - m
                            nc.vector.tensor_scalar(
                                out=bias1[:, qi:qi + 1], in0=mrow[:, qi:qi + 1],
                                scalar1=-1.0, scalar2=1.0,
                                op0=Alu.mult, op1=Alu.add,
                            )
                            # evacuate both halves: t = relu(z - (m-1)) -> fp16
                            nc.scalar.activation(
                                attn_t[qi][:, 0:hw], ps0[:, 0:hw], Relu,
                                bias=bias1[:, qi:qi + 1],
                            )
                            nc.scalar.activation(
                                attn_t[qi][:, hw:W], ps[:, 0:hw], Relu,
                                bias=bias1[:, qi:qi + 1],
                            )

                # sorted top-16 of the 32 candidates, per tile
                for qi in range(NT):
                    c32 = cand[:, qi * NC8:(qi + 1) * NC8]
                    s16 = sort16[:, qi * NCAND:(qi + 1) * NCAND]
                    scr = smpool.tile([P, NC8], F32, tag="scr")
                    nc.vector.max(out=s16[:, 0:8], in_=c32)
                    nc.vector.match_replace(
                        out=scr[:], in_to_replace=s16[:, 0:8], in_values=c32,
                        imm_value=-1e9,
                    )
                    nc.vector.max(out=s16[:, 8:16], in_=scr[:])

                # stacked tau computation (fp32, exact): view [P, NT, NCAND]
                s3 = sort16.rearrange("p (t c) -> p t c", c=NCAND)
                cum = smpool.tile([P, NT * NCAND], F32, tag="cum")
                cum3 = cum.rearrange("p (t c) -> p t c", c=NCAND)
                nc.vector.tensor_copy(cum[:], sort16[:])
                tmp = smpool.tile([P, NT * NCAND], F32, tag="tmp")
                tmp3 = tmp.rearrange("p (t c) -> p t c", c=NCAND)
                src, dst = cum3, tmp3
                srcf, dstf = cum, tmp
                for d in (1, 2, 4, 8):
                    nc.vector.tensor_tensor(
                        out=dst[:, :, d:], in0=src[:, :, d:], in1=src[:, :, :NCAND - d],
                        op=Alu.add,
                    )
                    nc.vector.tensor_copy(dst[:, :, 0:d], src[:, :, 0:d])
                    src, dst = dst, src
                    srcf, dstf = dstf, srcf
                # src now holds cumsum
                # cond = (1 + k*v - S) > 0
                u = smpool.tile([P, NT * NCAND], F32, tag="u")
                nc.vector.tensor_tensor(out=u[:], in0=sort16[:], in1=kvec_sb[:], op=Alu.mult)
                nc.vector.tensor_tensor(out=u[:], in0=u[:], in1=srcf[:], op=Alu.subtract)
                cnd = smpool.tile([P, NT * NCAND], F32, tag="cnd")
                nc.vector.tensor_scalar(
                    out=cnd[:], in0=u[:], scalar1=-1.0, scalar2=None, op0=Alu.is_gt,
                )
                # S_kz = sum(cnd * v); kz = sum(cnd)
                pv = smpool.tile([P, NT * NCAND], F32, tag="pv")
                nc.vector.tensor_tensor(out=pv[:], in0=cnd[:], in1=sort16[:], op=Alu.mult)
                skz = smpool.tile([P, NT], F32, tag="skz")
                nc.vector.tensor_reduce(
                    skz[:], pv.rearrange("p (t c) -> p t c", c=NCAND),
                    axis=mybir.AxisListType.X, op=Alu.add,
                )
                kz = smpool.tile([P, NT], F32, tag="kz")
                nc.vector.tensor_reduce(
                    kz[:], cnd.rearrange("p (t c) -> p t c", c=NCAND),
                    axis=mybir.AxisListType.X, op=Alu.add,
                )
                rkz = smpool.tile([P, NT], F32, tag="rkz")
                nc.vector.reciprocal(rkz[:], kz[:])
                # delta = (S-1)/kz - (m-1) ; store negdelta = -delta
                nc.vector.tensor_scalar(
                    out=skz[:], in0=skz[:], scalar1=-1.0, scalar2=None, op0=Alu.add,
                )
                tauz = smpool.tile([P, NT], F32, tag="tauz")
                nc.vector.tensor_tensor(out=tauz[:], in0=skz[:], in1=rkz[:], op=Alu.mult)
                ndelta = smpool.tile([P, NT], F32, tag="ndelta")
                # ndelta = (m - 1) - tau = -(bias1) ... bias1 = 1-m so m-1 = -bias1
                nc.vector.tensor_tensor(out=ndelta[:], in0=bias1[:], in1=tauz[:], op=Alu.add)
                nc.vector.tensor_scalar(
                    out=ndelta[:], in0=ndelta[:], scalar1=-1.0, scalar2=None, op0=Alu.mult,
                )

                # attn = relu(t - delta), PE-transpose blocks, attn.T @ v
                for ohalf in range(1, -1, -1):
                    psum_o = opool.tile([HD, L // 2], F32, tag="po",
                                        name=f"po{h}_{ohalf}")
                    for qi in range(8 * ohalf + 7, 8 * ohalf - 1, -1):
                        nc.vector.tensor_scalar(
                            out=attn_t[qi][:], in0=attn_t[qi][:],
                            scalar1=ndelta[:, qi:qi + 1], scalar2=0.0,
                            op0=Alu.add, op1=Alu.max,
                        )
                        for g in range(0, qi + 1, 4):
                            gend = min(qi + 1, g + 4)
                            gw = P * (gend - g)
                            pt = tpsum.tile([P, 4 * P], F16, tag="atr",
                                            name=f"atr{h}_{qi}_{g}")
                            for c in range(g, gend):
                                nc.tensor.transpose(
                                    pt[:, P * (c - g):P * (c - g + 1)],
                                    attn_t[qi][:, P * c:P * (c + 1)],
                                    identh_sb[:])
                            nc.vector.tensor_copy(
                                attnT_t[qi][:, P * g:P * g + gw], pt[:, :gw])
                            for c in range(g, gend):
                                nc.tensor.matmul(
                                    psum_o[:, P * (qi - 8 * ohalf):P * (qi - 8 * ohalf + 1)],
                                    v2[:, P * c:P * (c + 1)][:, hsl],
                                    attnT_t[qi][:, P * c:P * (c + 1)],
                                    start=(c == 0), stop=(c == qi),
                                )
                    nc.scalar.activation(
                        hoT2[HD * h:HD * (h + 1),
                             (L // 2) * ohalf:(L // 2) * (ohalf + 1)],
                        psum_o[:], Copy)

        # single fp16 output DMA [128, 2048]
        nc.sync.dma_start(ho, hoT2[:])


_NC_CACHE = {}


def _get_state():
    if "state" in _NC_CACHE:
        return _NC_CACHE["state"]

    import jax
    import jax.numpy as jnp
    from jax.sharding import Mesh, PartitionSpec

    import warnings
    with warnings.catch_warnings():
        warnings.simplefilter("ignore", DeprecationWarning)
        from jax.experimental.shard_map import shard_map

    nc = _build_program()
    b2j.install_neuronx_cc_hook()

    devices = jax.devices()[:8]
    mesh = Mesh(np.asarray(devices), ("core",))
    PC = PartitionSpec("core")

    # ----- stage 1: unpack + all_gather + per-core slicing, on device -----
    def _stage1(p_local):
        pa = jax.lax.all_gather(p_local[0], "core", axis=0)  # [8, NPK] f16
        cid = jax.lax.axis_index("core")
        bb = cid // 4
        j2 = cid % 4
        xall = pa[:, :XSZ].reshape(8, 512, 512)
        xb = jax.lax.dynamic_slice(xall, (4 * bb, 0, 0), (4, 512, 512))
        xt = xb.reshape(L, D).T                            # [512, 2048] f16
        ws = []
        for wi in range(3):
            wf = pa[:, XSZ + wi * WSZ:XSZ + (wi + 1) * WSZ].reshape(D, D)
            wsl = jax.lax.dynamic_slice(wf, (P * j2, 0), (P, D))
            ws.append(wsl.T)                               # [512, 128] f16
        row = jax.lax.broadcasted_iota(jnp.int32, (P, P), 0)
        col = jax.lax.broadcasted_iota(jnp.int32, (P, P), 1)
        dmask = jnp.where(col > row, jnp.float16(NEG), jnp.float16(0.0))
        identh = jnp.eye(P, dtype=jnp.float16)
        kvec = jnp.broadcast_to(
            jnp.tile(jnp.arange(1, NCAND + 1, dtype=jnp.float32), NT)[None, :],
            (P, NT * NCAND))
        zho = jnp.zeros((P, L), jnp.float16)
        return xt, ws[0], ws[1], ws[2], dmask, identh, kvec, zho

    stage1 = jax.jit(shard_map(
        _stage1, mesh=mesh, in_specs=(PC,), out_specs=(PC,) * 8, check_rep=False))

    # ----- stage 2: the Bass NEFF as a custom call on device arrays -----
    partition_name = nc.partition_id_tensor.name if nc.partition_id_tensor else None
    in_names, out_names, out_avals = [], [], []
    for alloc in nc.m.functions[0].allocations:
        if not isinstance(alloc, mybir.MemoryLocationSet):
            continue
        name = alloc.memorylocations[0].name
        if alloc.kind == "ExternalInput":
            if name != partition_name:
                in_names.append(name)
        elif alloc.kind == "ExternalOutput":
            out_names.append(name)
            out_avals.append(jax.core.ShapedArray(
                tuple(alloc.tensor_shape), mybir.dt.np(alloc.dtype)))
    n_params = len(in_names)
    n_outs = len(out_names)
    in_names_all = in_names + out_names
    if partition_name is not None:
        in_names_all.append(partition_name)
    donate = tuple(range(n_params, n_params + n_outs))

    def _body(*args):
        operands = list(args)
        if partition_name is not None:
            operands.append(b2j.partition_id_tensor())
        outs = b2j._bass_exec_p.bind(
            *operands,
            out_avals=tuple(out_avals),
            in_names=tuple(in_names_all),
            out_names=tuple(out_names),
            lowering_input_output_aliases=(),
            sim_require_finite=True,
            sim_require_nnan=True,
            nc=nc,
        )
        return tuple(outs)

    stage2 = jax.jit(
        shard_map(_body, mesh=mesh, in_specs=(PC,) * (n_params + n_outs),
                  out_specs=(PC,) * n_outs, check_rep=False),
        donate_argnums=donate, keep_unused=True)

    state = {"nc": nc, "stage1": stage1, "stage2": stage2,
             "in_names": in_names, "out_names": out_names}
    _NC_CACHE["state"] = state
    return state


def _pack_inputs(x, W_q, W_k, W_v):
    packed = np.empty((8, NPK), np.float16)
    packed[:, :XSZ] = x.reshape(8, XSZ).astype(np.float16)
    # 1/sqrt(head_dim) score scale folded into the q projection
    packed[:, XSZ:XSZ + WSZ] = (W_q * np.float32(0.125)).astype(np.float16).reshape(8, WSZ)
    packed[:, XSZ + WSZ:XSZ + 2 * WSZ] = W_k.astype(np.float16).reshape(8, WSZ)
    packed[:, XSZ + 2 * WSZ:] = W_v.astype(np.float16).reshape(8, WSZ)
    return packed


_POOL = None


def _project_out(ho_np, W_o):
    # ho_np [1024, 2048] fp16: rows 128c:128c+128 = core c's two heads' out.T
    global _POOL
    if _POOL is None:
        from concurrent.futures import ThreadPoolExecutor
        _POOL = ThreadPoolExecutor(2)
    out = np.empty((2, L, D), np.float32)
    Wo32 = np.asarray(W_o, dtype=np.float32)

    def _one(bb):
        A = ho_np[D * bb:D * (bb + 1)].astype(np.float32)   # [512, 2048]
        out[bb] = (Wo32 @ A).T

    list(_POOL.map(_one, range(2)))
    return out


def _kernel_traced(x, W_q, W_k, W_v, W_o):
    """Debug/profiling path through run_bass_kernel_spmd (KERNEL_TRACE=1)."""
    nc = _get_state()["nc"]
    dmask_np = np.where(
        np.arange(P)[None, :] > np.arange(P)[:, None],
        np.float16(NEG), np.float16(0.0)).astype(np.float16)
    identh_np = np.eye(P, dtype=np.float16)
    kvec_np = np.broadcast_to(
        np.tile(np.arange(1, NCAND + 1, dtype=np.float32), NT)[None, :],
        (P, NT * NCAND)).copy()
    in_maps = []
    for c in range(8):
        bb = c // 4
        hs = slice(P * (c % 4), P * (c % 4 + 1))
        in_maps.append({
            "xt": np.ascontiguousarray(x[bb].T).astype(np.float16),
            "wq": np.ascontiguousarray((W_q[hs] * np.float32(0.125)).T).astype(np.float16),
            "wk": np.ascontiguousarray(W_k[hs].T).astype(np.float16),
            "wv": np.ascontiguousarray(W_v[hs].T).astype(np.float16),
            "dmask": dmask_np, "identh": identh_np, "kvec": kvec_np,
        })
    try:
        res = run_bass_kernel_spmd(nc, in_maps, core_ids=list(range(8)), trace=True)
    except Exception:
        res = run_bass_kernel_spmd(nc, in_maps, core_ids=list(range(8)), trace=False)
    _NC_CACHE["last_results"] = res
    ho_np = np.concatenate([res.results[c]["ho"] for c in range(8)], axis=0)
    return _project_out(ho_np, W_o)


def kernel(x, W_q, W_k, W_v, W_o):
    x = np.asarray(x, dtype=np.float32)
    W_q = np.asarray(W_q, dtype=np.float32)
    W_k = np.asarray(W_k, dtype=np.float32)
    W_v = np.asarray(W_v, dtype=np.float32)
    W_o = np.asarray(W_o, dtype=np.float32)

    if bool(int(os.environ.get("KERNEL_TRACE", "0"))):
        return _kernel_traced(x, W_q, W_k, W_v, W_o)

    st = _get_state()
    packed = _pack_inputs(x, W_q, W_k, W_v)
    s1 = st["stage1"](packed)
    ho = st["stage2"](*s1)[0]
    ho_np = np.asarray(ho)  # [1024, 2048] fp16
    return _project_out(ho_np, W_o)
